# revision 4
# baseline (speedup 1.0000x reference)
"""Banded (lookahead) cross-attention on 8 Trainium2 NeuronCores.

Reference computation (B=4, T=2048, D=1024, H=16, hd=64):
    Q = query @ Wq.T + bq ; K = key_value @ Wk.T + bk ; V = key_value @ Wv.T + bv
    scores = Q K^T / sqrt(hd), masked to j <= i + lookahead
    out = softmax(scores) V, concat heads, @ Wo.T + bo

Sharding: 8 cores = (batch b = c//2) x (head-half = c%2, 8 heads each).
Each core computes a full [T, D] partial of the output projection for its
8 heads; host sums the two partials per batch and adds bo.

Per-core kernel (all matmuls f32r except the hd=64 score matmul in bf16):
  A: Q^T[e,t] = (Wq_s^T)^T-tiled @ query^T-tiled + bq     (psum acc over D)
  B: K^T likewise
  C: V[t,e] in a [128, 8*65] per-t-tile layout, head h at cols 65h..65h+64,
     col 65h+64 = 1.0 (ones column -> softmax denominators for free)
  D: per head, j-block outer / i-chunk inner flash attention without
     max-subtraction (scores bounded), S^T layout [j, i]:
       S^T = K_h^T-block.T @ Q_h^T   (bf16, c=64)
       P^T = exp(S^T / 8) (*banded mask on boundary chunks)
       O^T[65, i] += V_aug.T @ P^T   (f32r, c=128; row 64 = denominator)
     then A^T[e, i] = O^T[0:64] * recip(O^T[64]) broadcast
  E: outT_partial = (Wo_s^T).T @ A^T   [D, T]
Host: out[b] = (outT[2b] + outT[2b+1]).T + bo
"""

import sys

for _p in ("/opt/trn_rl_repo", "/opt/pypackages"):
    if _p not in sys.path:
        sys.path.append(_p)

import numpy as np

import concourse.bass as bass
import concourse.tile as tile
from concourse import bacc, mybir
from concourse.bass_utils import run_bass_kernel_spmd

F32 = mybir.dt.float32
F32R = mybir.dt.float32r
BF16 = mybir.dt.bfloat16
AF = mybir.ActivationFunctionType

B, T, D = 4, 2048, 1024
H, HD = 16, 64
H_LOC = 8                    # heads per core
E_LOC = H_LOC * HD           # 512 projected dims per core
NJB = T // 128               # 16 j-blocks
NIC = T // 512               # 4 i-chunks
NDT = D // 128               # 8 contraction tiles
NET = E_LOC // 128           # 4 e-tiles
SCALE = HD ** -0.5
VW = H_LOC * (HD + 1)        # 520: per-t-tile V layout width

_CACHE = {}


def _band_chunks(L):
    """Per j-block list of (ic, delta|None). delta=None -> dense chunk."""
    per_jb = []
    deltas = []
    for jb in range(NJB):
        j0 = 128 * jb
        lst = []
        for ic in range(NIC):
            i0 = 512 * ic
            if i0 + 511 + L < j0:
                continue                      # fully masked
            if j0 + 127 <= i0 + L:
                lst.append((ic, None))        # fully dense
            else:
                d = j0 - L - i0
                lst.append((ic, d))
                if d not in deltas:
                    deltas.append(d)
        per_jb.append(lst)
    return per_jb, sorted(deltas)


def _build(L):
    per_jb, deltas = _band_chunks(L)
    dpos = {d: k for k, d in enumerate(deltas)}
    nmask = max(1, len(deltas))

    nc = bacc.Bacc("TRN2", target_bir_lowering=False, debug=False)
    xqT = nc.dram_tensor("xqT", [D, T], F32R, kind="ExternalInput").ap()
    xkvT = nc.dram_tensor("xkvT", [D, T], F32R, kind="ExternalInput").ap()
    wqT = nc.dram_tensor("wqT", [D, E_LOC], F32R, kind="ExternalInput").ap()
    wkT = nc.dram_tensor("wkT", [D, E_LOC], F32R, kind="ExternalInput").ap()
    wvT = nc.dram_tensor("wvT", [D, VW], F32R, kind="ExternalInput").ap()
    woT = nc.dram_tensor("woT", [E_LOC, D], F32R, kind="ExternalInput").ap()
    bq4 = nc.dram_tensor("bq4", [128, NET], F32, kind="ExternalInput").ap()
    bk4 = nc.dram_tensor("bk4", [128, NET], F32, kind="ExternalInput").ap()
    bv_row = nc.dram_tensor("bv_row", [1, VW], F32R, kind="ExternalInput").ap()
    ones1 = nc.dram_tensor("ones1", [1, 128], F32R, kind="ExternalInput").ap()
    masks = nc.dram_tensor("masks", [128, nmask * 512], F32,
                           kind="ExternalInput").ap()
    outT = nc.dram_tensor("outT", [D, T], F32, kind="ExternalOutput").ap()

    with tile.TileContext(nc) as tc:
        with tc.tile_pool(name="small", bufs=1) as small, \
             tc.tile_pool(name="at", bufs=1) as at_pool:
            bq_sb = small.tile([128, NET], F32, tag="bq")
            bk_sb = small.tile([128, NET], F32, tag="bk")
            bv_sb = small.tile([1, VW], F32R, tag="bv")
            on_sb = small.tile([1, 128], F32R, tag="on")
            nc.sync.dma_start(bq_sb[:], bq4[:])
            nc.sync.dma_start(bk_sb[:], bk4[:])
            nc.sync.dma_start(bv_sb[:], bv_row[:])
            nc.sync.dma_start(on_sb[:], ones1[:])

            aT = [at_pool.tile([128, T], F32R, tag=f"at{i}", name=f"at{i}") for i in range(NET)]

            with tc.tile_pool(name="big", bufs=1) as big:
                qT = [big.tile([128, T], BF16, tag=f"qt{i}", name=f"qt{i}") for i in range(NET)]
                kT = [big.tile([128, T], BF16, tag=f"kt{i}", name=f"kt{i}") for i in range(NET)]
                v_sb = [big.tile([128, VW], F32R, tag=f"v{i}", name=f"v{i}") for i in range(NJB)]
                mk_sb = big.tile([128, nmask * 512], F32, tag="mk")
                nc.sync.dma_start(mk_sb[:], masks[:])

                # ---- phases A (Q^T) and B (K^T) ----
                for (x_ap, w_ap, b_sb, dst) in ((xqT, wqT, bq_sb, qT),
                                                (xkvT, wkT, bk_sb, kT)):
                    with tc.tile_pool(name="xs", bufs=1) as xs_pool, \
                         tc.tile_pool(name="ws", bufs=1) as ws_pool, \
                         tc.tile_pool(name="pps", bufs=4, space="PSUM") as pps:
                        w_sb = [ws_pool.tile([128, E_LOC], F32R, tag=f"w{d}", name=f"w{d}")
                                for d in range(NDT)]
                        x_sb = [xs_pool.tile([128, T], F32R, tag=f"x{d}", name=f"xA{d}")
                                for d in range(NDT)]
                        for d in range(NDT):
                            nc.sync.dma_start(w_sb[d][:], w_ap[128 * d:128 * (d + 1), :])
                            nc.sync.dma_start(x_sb[d][:], x_ap[128 * d:128 * (d + 1), :])
                        for et in range(NET):
                            for t in range(NIC):
                                ps = pps.tile([128, 512], F32, tag="p")
                                for d in range(NDT):
                                    nc.tensor.matmul(
                                        ps[:],
                                        w_sb[d][:, 128 * et:128 * (et + 1)],
                                        x_sb[d][:, 512 * t:512 * (t + 1)],
                                        start=(d == 0), stop=(d == NDT - 1))
                                nc.scalar.activation(
                                    dst[et][:, 512 * t:512 * (t + 1)], ps[:],
                                    AF.Identity, bias=b_sb[:, et:et + 1])

                # ---- phase C (V in [128, 520] per-t-tile layout) ----
                with tc.tile_pool(name="xs2", bufs=1) as xs_pool, \
                     tc.tile_pool(name="wv", bufs=1) as wv_pool, \
                     tc.tile_pool(name="vps", bufs=2, space="PSUM") as vps, \
                     tc.tile_pool(name="vbs", bufs=2, space="PSUM") as vbs:
                    x_sb = [xs_pool.tile([128, T], F32R, tag=f"x{d}", name=f"xC{d}")
                            for d in range(NDT)]
                    for d in range(NDT):
                        nc.sync.dma_start(x_sb[d][:], xkvT[128 * d:128 * (d + 1), :])
                    wv_sb = [wv_pool.tile([128, VW], F32R, tag=f"wv{d}", name=f"wv{d}")
                             for d in range(NDT)]
                    for d in range(NDT):
                        nc.sync.dma_start(wv_sb[d][:], wvT[128 * d:128 * (d + 1), :])
                    for tt in range(NJB):
                        ps = vps.tile([128, 512], F32, tag="p")
                        pb = vbs.tile([128, VW - 512], F32, tag="pb")
                        for d in range(NDT):
                            nc.tensor.matmul(
                                ps[:], x_sb[d][:, 128 * tt:128 * (tt + 1)],
                                wv_sb[d][:, 0:512],
                                start=(d == 0), stop=False)
                            nc.tensor.matmul(
                                pb[:], x_sb[d][:, 128 * tt:128 * (tt + 1)],
                                wv_sb[d][:, 512:VW],
                                start=(d == 0), stop=False)
                        # bias row via ones (adds bv to cols, 1.0 to ones cols)
                        nc.tensor.matmul(ps[:], on_sb[:, 0:128],
                                         bv_sb[:, 0:512], start=False, stop=True)
                        nc.tensor.matmul(pb[:], on_sb[:, 0:128],
                                         bv_sb[:, 512:VW], start=False, stop=True)
                        nc.scalar.activation(v_sb[tt][:, 0:512], ps[:], AF.Copy)
                        nc.scalar.activation(v_sb[tt][:, 512:VW], pb[:], AF.Copy)

                # ---- phase D (banded flash attention, S^T layout) ----
                with tc.tile_pool(name="pt", bufs=4) as pt_pool, \
                     tc.tile_pool(name="dv", bufs=4) as dv_pool, \
                     tc.tile_pool(name="sps", bufs=3, space="PSUM") as sps, \
                     tc.tile_pool(name="ops", bufs=4, space="PSUM") as ops:
                    for h in range(H_LOC):
                        et, r0 = h // 2, 64 * (h % 2)
                        flat = [(jb, ic, dlt) for jb in range(NJB)
                                for (ic, dlt) in per_jb[jb]]
                        first_jb = {}
                        last_jb = {}
                        for jb, ic, _ in flat:
                            first_jb.setdefault(ic, jb)
                            last_jb[ic] = jb
                        ot = {ic: ops.tile([65, 512], F32, tag="ot", name=f"ot{h}_{ic}")
                              for ic in range(NIC)}
                        pts = {}
                        DEPTH = 3

                        def mm2(n):
                            jb, ic, _ = flat[n]
                            nc.tensor.matmul(
                                ot[ic][:], v_sb[jb][:, VW8 * h:VW8 * h + HD + 1],
                                pts.pop(n)[:],
                                start=(jb == first_jb[ic]),
                                stop=(jb == last_jb[ic]))

                        VW8 = HD + 1
                        for n in range(len(flat)):
                            jb, ic, dlt = flat[n]
                            st = sps.tile([128, 512], F32, tag="st")
                            nc.tensor.matmul(
                                st[:],
                                kT[et][r0:r0 + 64, 128 * jb:128 * (jb + 1)],
                                qT[et][r0:r0 + 64, 512 * ic:512 * (ic + 1)],
                                start=True, stop=True)
                            pt = pt_pool.tile([128, 512], F32R, tag="pt")
                            nc.scalar.activation(pt[:], st[:], AF.Exp, scale=SCALE)
                            if dlt is not None:
                                k = dpos[dlt]
                                nc.vector.tensor_tensor(
                                    pt[:], pt[:], mk_sb[:, 512 * k:512 * (k + 1)],
                                    mybir.AluOpType.mult)
                            pts[n] = pt
                            if n >= DEPTH:
                                mm2(n - DEPTH)
                        for n in range(max(0, len(flat) - DEPTH), len(flat)):
                            mm2(n)
                        # divide by denominators -> A^T rows
                        for ic in range(NIC):
                            r = dv_pool.tile([1, 512], F32, tag="r")
                            nc.vector.reciprocal(r[:], ot[ic][64:65, :])
                            rb = dv_pool.tile([64, 512], F32, tag="rb")
                            nc.gpsimd.partition_broadcast(rb[:], r[:])
                            nc.vector.tensor_tensor(
                                aT[et][r0:r0 + 64, 512 * ic:512 * (ic + 1)],
                                ot[ic][0:64, :], rb[:], mybir.AluOpType.mult)

            # ---- phase E (output projection partial, [D, T]) ----
            with tc.tile_pool(name="wo", bufs=1) as wo_pool, \
                 tc.tile_pool(name="os", bufs=4) as os_pool, \
                 tc.tile_pool(name="eps", bufs=4, space="PSUM") as eps:
                wo_sb = [wo_pool.tile([128, D], F32R, tag=f"wo{e}", name=f"wo{e}")
                         for e in range(NET)]
                for e in range(NET):
                    nc.sync.dma_start(wo_sb[e][:], woT[128 * e:128 * (e + 1), :])
                for do in range(NDT):
                    for ic in range(NIC):
                        ps = eps.tile([128, 512], F32, tag="p")
                        for e in range(NET):
                            nc.tensor.matmul(
                                ps[:], wo_sb[e][:, 128 * do:128 * (do + 1)],
                                aT[e][:, 512 * ic:512 * (ic + 1)],
                                start=(e == 0), stop=(e == NET - 1))
                        o = os_pool.tile([128, 512], F32, tag="o")
                        nc.scalar.activation(o[:], ps[:], AF.Copy)
                        nc.sync.dma_start(
                            outT[128 * do:128 * (do + 1), 512 * ic:512 * (ic + 1)],
                            o[:])

    nc.compile()
    return nc, deltas


def _prep_core(query, key_value, Wq, bq, Wk, bk, Wv, bv, Wo, c, deltas, L):
    b, half = c // 2, c % 2
    hs = E_LOC * half
    f32 = np.float32
    xqT = np.ascontiguousarray(query[b].T, dtype=f32)
    xkvT = np.ascontiguousarray(key_value[b].T, dtype=f32)
    wqT = np.ascontiguousarray(Wq[hs:hs + E_LOC].T, dtype=f32)
    wkT = np.ascontiguousarray(Wk[hs:hs + E_LOC].T, dtype=f32)
    wvT = np.zeros((D, VW), dtype=f32)
    bv_row = np.zeros((1, VW), dtype=f32)
    for h in range(H_LOC):
        wvT[:, (HD + 1) * h:(HD + 1) * h + HD] = \
            Wv[hs + HD * h:hs + HD * (h + 1)].T
        bv_row[0, (HD + 1) * h:(HD + 1) * h + HD] = bv[hs + HD * h:hs + HD * (h + 1)]
        bv_row[0, (HD + 1) * h + HD] = 1.0
    woT = np.ascontiguousarray(Wo[:, hs:hs + E_LOC].T, dtype=f32)
    bq4 = np.ascontiguousarray(bq[hs:hs + E_LOC].reshape(NET, 128).T, dtype=f32)
    bk4 = np.ascontiguousarray(bk[hs:hs + E_LOC].reshape(NET, 128).T, dtype=f32)
    ones1 = np.ones((1, 128), dtype=f32)
    nmask = max(1, len(deltas))
    masks = np.zeros((128, nmask * 512), dtype=f32)
    jr = np.arange(128)[:, None]
    ir = np.arange(512)[None, :]
    for k, d in enumerate(deltas):
        masks[:, 512 * k:512 * (k + 1)] = (jr <= ir - d).astype(f32)
    return {"xqT": xqT, "xkvT": xkvT, "wqT": wqT, "wkT": wkT, "wvT": wvT,
            "woT": woT, "bq4": bq4, "bk4": bk4, "bv_row": bv_row,
            "ones1": ones1, "masks": masks}


def kernel(query, key_value, Wq, bq, Wk, bk, Wv, bv, Wo, bo, lookahead,
           _trace=False):
    L = int(lookahead)
    if L not in _CACHE:
        _CACHE[L] = _build(L)
    nc, deltas = _CACHE[L]

    args = [np.asarray(a, dtype=np.float32) for a in
            (query, key_value, Wq, bq, Wk, bk, Wv, bv, Wo)]
    in_maps = [_prep_core(*args, c, deltas, L) for c in range(8)]
    res = run_bass_kernel_spmd(nc, in_maps, core_ids=list(range(8)),
                               trace=_trace)
    bo = np.asarray(bo, dtype=np.float32)
    out = np.empty((B, T, D), dtype=np.float32)
    for b in range(B):
        pT = res.results[2 * b]["outT"] + res.results[2 * b + 1]["outT"]
        out[b] = pT.T + bo[None, :]
    if _trace:
        kernel.last_exec_time_ns = res.exec_time_ns
    return out


# revision 8
# speedup vs baseline: 1.4730x; 1.4730x over previous
"""Banded (lookahead) cross-attention on 8 Trainium2 NeuronCores.

Reference computation (B=4, T=2048, D=1024, H=16, hd=64):
    Q = query @ Wq.T + bq ; K = key_value @ Wk.T + bk ; V = key_value @ Wv.T + bv
    scores = Q K^T / sqrt(hd), masked to j <= i + lookahead
    out = softmax(scores) V, concat heads, @ Wo.T + bo

Sharding: 8 cores = (batch b = c//2) x (head-half = c%2, 8 heads each).
Each core computes a full [T, D] partial of the output projection for its
8 heads; host sums the two partials per batch and adds bo.

Per-core phases (x/weight slabs bf16; V/P/A/Wo f32r; psum f32):
  A: Q^T[e,t] psum-accumulated over D, + bq, stored bf16
  B: K^T likewise
  C: V[t,e] + bv via ones-row matmul, stored f32r in a [128, 8*65] layout
     (head h at cols 65h..65h+64, col 65h+64 = 1.0 -> softmax denominators)
  D: per (head, i-chunk) banded flash attention without max-subtraction
     (scores are bounded for these input scales), S^T layout [j, i]; two
     j-blocks paired per psum tile so each ACT exp covers [128, 1024];
     boundary chunks delta-trimmed; denominator row comes out of the ones
     column; divide via DVE recip + gpsimd partition broadcast
  E: outT_partial[D, T] = Wo_s^T.T @ A^T
Host: out[b] = (outT[2b] + outT[2b+1]).T + bo
"""

import sys

for _p in ("/opt/trn_rl_repo", "/opt/pypackages"):
    if _p not in sys.path:
        sys.path.append(_p)

import numpy as np
import ml_dtypes

import concourse.bass as bass
import concourse.tile as tile
from concourse import bacc, mybir
from concourse.bass_utils import run_bass_kernel_spmd

F32 = mybir.dt.float32
F32R = mybir.dt.float32r
BF16 = mybir.dt.bfloat16
AF = mybir.ActivationFunctionType
MUL = mybir.AluOpType.mult

B, T, D = 4, 2048, 1024
H, HD = 16, 64
H_LOC = 8                    # heads per core
E_LOC = H_LOC * HD           # 512 projected dims per core
NJB = T // 128               # 16 j-blocks
NIC = T // 512               # 4 i-chunks
NDT = D // 128               # 8 contraction tiles
NET = E_LOC // 128           # 4 e-tiles
SCALE = HD ** -0.5
VW = H_LOC * (HD + 1)        # 520 v_sb layout width
VH = HD + 1                  # 65

_CACHE = {}


def _groups(L):
    """Per i-chunk: list of (jb, delta, masked); delta = first valid column
    offset inside the 512-wide chunk (0 for dense)."""
    out = []
    deltas = set()
    for ic in range(NIC):
        i0 = 512 * ic
        lst = []
        for jb in range(NJB):
            j0 = 128 * jb
            if i0 + 511 + L < j0:
                break                          # fully masked from here on
            if j0 + 127 <= i0 + L:
                lst.append((jb, 0, False))     # dense
            else:
                d = j0 - L - i0
                lst.append((jb, max(d, 0), True))
                deltas.add(d)
        out.append(lst)
    return out, sorted(deltas)


def _build(L):
    groups, deltas = _groups(L)
    dpos = {d: k for k, d in enumerate(deltas)}
    nmask = max(1, len(deltas))

    nc = bacc.Bacc("TRN2", target_bir_lowering=False, debug=False)
    xqT = nc.dram_tensor("xqT", [D, T], BF16, kind="ExternalInput").ap()
    xkvT = nc.dram_tensor("xkvT", [D, T], BF16, kind="ExternalInput").ap()
    wqT = nc.dram_tensor("wqT", [D, E_LOC], BF16, kind="ExternalInput").ap()
    wkT = nc.dram_tensor("wkT", [D, E_LOC], BF16, kind="ExternalInput").ap()
    wvT = nc.dram_tensor("wvT", [D, E_LOC], BF16, kind="ExternalInput").ap()
    woT = nc.dram_tensor("woT", [E_LOC, D], F32R, kind="ExternalInput").ap()
    bq4 = nc.dram_tensor("bq4", [128, NET], F32, kind="ExternalInput").ap()
    bk4 = nc.dram_tensor("bk4", [128, NET], F32, kind="ExternalInput").ap()
    bv_row = nc.dram_tensor("bv_row", [1, E_LOC], BF16, kind="ExternalInput").ap()
    ones1 = nc.dram_tensor("ones1", [1, 128], BF16, kind="ExternalInput").ap()
    ones8 = nc.dram_tensor("ones8", [128, H_LOC], F32, kind="ExternalInput").ap()
    masks = nc.dram_tensor("masks", [128, nmask * 512], F32,
                           kind="ExternalInput").ap()
    outT = nc.dram_tensor("outT", [D, T], F32, kind="ExternalOutput").ap()

    with tile.TileContext(nc) as tc:
        with tc.tile_pool(name="small", bufs=1) as small, \
             tc.tile_pool(name="persist", bufs=1) as persist:
          with tc.tile_pool(name="slabs", bufs=1) as slabs:

            # --- DMA issue order == first-need order ---
            wq_sb = [slabs.tile([128, E_LOC], BF16, tag=f"wq{d}", name=f"wq{d}")
                     for d in range(NDT)]
            for d in range(NDT):
                nc.sync.dma_start(wq_sb[d][:], wqT[128 * d:128 * (d + 1), :])
            bq_sb = small.tile([128, NET], F32, tag="bq")
            bk_sb = small.tile([128, NET], F32, tag="bk")
            bv_sb = small.tile([1, E_LOC], BF16, tag="bv")
            on_sb = small.tile([1, 128], BF16, tag="on")
            on8_sb = small.tile([128, H_LOC], F32, tag="on8")
            nc.sync.dma_start(bq_sb[:], bq4[:])
            nc.sync.dma_start(bk_sb[:], bk4[:])
            nc.sync.dma_start(bv_sb[:], bv_row[:])
            nc.sync.dma_start(on_sb[:], ones1[:])
            nc.sync.dma_start(on8_sb[:], ones8[:])
            # xq as (d, t)-tiles through a 16-slot rotating pool, t-major:
            # the first t-column is ready after ~1MB of DMA and slots recycle
            # as each t-column is consumed
            xq_sb = {}
            for t in range(NIC):
                for d in range(NDT):
                    xq_sb[(d, t)] = slabs.tile(
                        [128, 512], BF16, tag="xq", bufs=16, name=f"xq{d}_{t}")
                    nc.sync.dma_start(
                        xq_sb[(d, t)][:],
                        xqT[128 * d:128 * (d + 1), 512 * t:512 * (t + 1)])
            # prefetched for B/C:
            xkv_sb = [slabs.tile([128, T], BF16, tag=f"xkv{d}", name=f"xkv{d}")
                      for d in range(NDT)]
            wk_sb = [slabs.tile([128, E_LOC], BF16, tag=f"wk{d}", name=f"wk{d}")
                     for d in range(NDT)]
            wv_sb = [slabs.tile([128, E_LOC], BF16, tag=f"wv{d}", name=f"wv{d}")
                     for d in range(NDT)]
            for d in range(NDT):
                nc.sync.dma_start(xkv_sb[d][:], xkvT[128 * d:128 * (d + 1), :])
            for d in range(NDT):
                nc.sync.dma_start(wk_sb[d][:], wkT[128 * d:128 * (d + 1), :])
                nc.sync.dma_start(wv_sb[d][:], wvT[128 * d:128 * (d + 1), :])

            qT = [persist.tile([128, T], BF16, tag=f"qt{i}", name=f"qt{i}")
                  for i in range(NET)]
            kT = [persist.tile([128, T], BF16, tag=f"kt{i}", name=f"kt{i}")
                  for i in range(NET)]
            v_sb = [persist.tile([128, VW], F32R, tag=f"v{i}", name=f"v{i}")
                    for i in range(NJB)]
            aT = [persist.tile([128, T], F32R, tag=f"at{i}", name=f"at{i}")
                  for i in range(NET)]
            mk_sb = persist.tile([128, nmask * 512], F32, tag="mk")
            nc.sync.dma_start(mk_sb[:], masks[:])

            # ---- phase A: Q^T ----
            with tc.tile_pool(name="pps", bufs=4, space="PSUM") as pps:
                for t in range(NIC):
                    for et in range(NET):
                        ps = pps.tile([128, 512], F32, tag="p")
                        for d in range(NDT):
                            nc.tensor.matmul(
                                ps[:], wq_sb[d][:, 128 * et:128 * (et + 1)],
                                xq_sb[(d, t)][:],
                                start=(d == 0), stop=(d == NDT - 1))
                        nc.scalar.activation(
                            qT[et][:, 512 * t:512 * (t + 1)], ps[:],
                            AF.Identity, bias=bq_sb[:, et:et + 1])

                # ---- phase B: K^T ----
                for et in range(NET):
                    for t in range(NIC):
                        ps = pps.tile([128, 512], F32, tag="p")
                        for d in range(NDT):
                            nc.tensor.matmul(
                                ps[:], wk_sb[d][:, 128 * et:128 * (et + 1)],
                                xkv_sb[d][:, 512 * t:512 * (t + 1)],
                                start=(d == 0), stop=(d == NDT - 1))
                        nc.scalar.activation(
                            kT[et][:, 512 * t:512 * (t + 1)], ps[:],
                            AF.Identity, bias=bk_sb[:, et:et + 1])

                # ---- phase C: V (+ ones columns) ----
                for tt in range(NJB):
                    ps = pps.tile([128, 512], F32, tag="p")
                    for d in range(NDT):
                        nc.tensor.matmul(
                            ps[:], xkv_sb[d][:, 128 * tt:128 * (tt + 1)],
                            wv_sb[d][:], start=(d == 0), stop=False)
                    nc.tensor.matmul(ps[:], on_sb[:], bv_sb[:],
                                     start=False, stop=True)
                    # scatter per-head cols into the 65-strided layout
                    vv = v_sb[tt][:].rearrange("p (h w) -> p h w", w=VH)
                    nc.scalar.activation(
                        vv[:, :, 0:HD],
                        ps[:].rearrange("p (h w) -> p h w", w=HD), AF.Copy)
                    nc.scalar.activation(
                        vv[:, :, HD:VH],
                        on8_sb[:].rearrange("p (h w) -> p h w", w=1), AF.Copy)

          # ---- phase D: banded attention (Wo prefetches for phase E) ----
          with tc.tile_pool(name="wo", bufs=1) as wo_pool:
            with tc.tile_pool(name="pt", bufs=6) as pt_pool, \
                 tc.tile_pool(name="dv", bufs=4) as dv_pool, \
                 tc.tile_pool(name="sps", bufs=3, space="PSUM") as sps, \
                 tc.tile_pool(name="ops", bufs=2, space="PSUM") as ops:
                wo_sb = [wo_pool.tile([128, D], F32R, tag=f"wo{e}", name=f"wo{e}")
                         for e in range(NET)]
                for e in range(NET):
                    nc.sync.dma_start(wo_sb[e][:], woT[128 * e:128 * (e + 1), :])
                # flat pair-list across all heads/i-chunks for pipelining
                work = []   # (h, ic, pair) ; pair = [(jb, delta, masked)] x<=2
                for h in range(H_LOC):
                    for ic in range(NIC):
                        lst = groups[ic]
                        for k in range(0, len(lst), 2):
                            work.append((h, ic, lst[k:k + 2]))

                ot = {}           # (h, ic) -> psum tile
                pending = {}      # n -> pt tile
                DEPTH = 2

                def emit_mm2(n):
                    h, ic, pair = work[n]
                    pt = pending.pop(n)
                    lst = groups[ic]
                    for s, (jb, dlt, msk) in enumerate(pair):
                        nc.tensor.matmul(
                            ot[(h, ic)][:, dlt:512],
                            v_sb[jb][:, VH * h:VH * h + VH],
                            pt[:, 512 * s + dlt:512 * (s + 1)],
                            start=(jb == lst[0][0]),
                            stop=(jb == lst[-1][0]),
                            skip_group_check=True)
                    if pair[-1][0] == lst[-1][0]:
                        emit_div(h, ic)

                def emit_div(h, ic):
                    o = ot.pop((h, ic))
                    et, r0 = h // 2, 64 * (h % 2)
                    r = dv_pool.tile([1, 512], F32, tag="r")
                    nc.vector.reciprocal(r[:], o[64:65, :])
                    rb = dv_pool.tile([64, 512], F32, tag="rb")
                    nc.gpsimd.partition_broadcast(rb[:], r[:])
                    nc.vector.tensor_tensor(
                        aT[et][r0:r0 + 64, 512 * ic:512 * (ic + 1)],
                        o[0:64, :], rb[:], MUL)

                for n, (h, ic, pair) in enumerate(work):
                    if (h, ic) not in ot:
                        ot[(h, ic)] = ops.tile([65, 512], F32, tag="ot",
                                               name=f"ot{h}_{ic}")
                    et, r0 = h // 2, 64 * (h % 2)
                    st = sps.tile([128, 1024], F32, tag="st")
                    for s, (jb, dlt, msk) in enumerate(pair):
                        nc.tensor.matmul(
                            st[:, 512 * s + dlt:512 * (s + 1)],
                            kT[et][r0:r0 + 64, 128 * jb:128 * (jb + 1)],
                            qT[et][r0:r0 + 64, 512 * ic + dlt:512 * (ic + 1)],
                            start=True, stop=True)
                    pt = pt_pool.tile([128, 1024], F32R, tag="pt")
                    lo = pair[0][1]
                    hi = 512 * (len(pair) - 1) + 512
                    nc.scalar.activation(pt[:, lo:hi], st[:, lo:hi],
                                         AF.Exp, scale=SCALE)
                    for s, (jb, dlt, msk) in enumerate(pair):
                        if msk:
                            k = dpos[128 * jb - L - 512 * ic]
                            nc.vector.tensor_tensor(
                                pt[:, 512 * s + dlt:512 * (s + 1)],
                                pt[:, 512 * s + dlt:512 * (s + 1)],
                                mk_sb[:, 512 * k + dlt:512 * (k + 1)], MUL)
                    pending[n] = pt
                    if n >= DEPTH:
                        emit_mm2(n - DEPTH)
                for n in range(max(0, len(work) - DEPTH), len(work)):
                    emit_mm2(n)

            # ---- phase E: output projection partial ----
            with tc.tile_pool(name="os", bufs=4) as os_pool, \
                 tc.tile_pool(name="eps", bufs=4, space="PSUM") as eps:
                for do in range(NDT):
                    for ic in range(NIC):
                        ps = eps.tile([128, 512], F32, tag="ep")
                        for e in range(NET):
                            nc.tensor.matmul(
                                ps[:], wo_sb[e][:, 128 * do:128 * (do + 1)],
                                aT[e][:, 512 * ic:512 * (ic + 1)],
                                start=(e == 0), stop=(e == NET - 1))
                        o = os_pool.tile([128, 512], F32, tag="eo")
                        nc.scalar.activation(o[:], ps[:], AF.Copy)
                        nc.sync.dma_start(
                            outT[128 * do:128 * (do + 1),
                                 512 * ic:512 * (ic + 1)], o[:])

    nc.compile()
    return nc, deltas


def _prep_core(query, key_value, Wq, bq, Wk, bk, Wv, bv, Wo, c, deltas, L):
    b, half = c // 2, c % 2
    hs = E_LOC * half
    f32, bf16 = np.float32, ml_dtypes.bfloat16
    xqT = np.ascontiguousarray(query[b].T).astype(bf16)
    xkvT = np.ascontiguousarray(key_value[b].T).astype(bf16)
    wqT = np.ascontiguousarray(Wq[hs:hs + E_LOC].T).astype(bf16)
    wkT = np.ascontiguousarray(Wk[hs:hs + E_LOC].T).astype(bf16)
    wvT = np.ascontiguousarray(Wv[hs:hs + E_LOC].T).astype(bf16)
    bv_row = bv[hs:hs + E_LOC].reshape(1, E_LOC).astype(bf16)
    woT = np.ascontiguousarray(Wo[:, hs:hs + E_LOC].T, dtype=f32)
    bq4 = np.ascontiguousarray(bq[hs:hs + E_LOC].reshape(NET, 128).T, dtype=f32)
    bk4 = np.ascontiguousarray(bk[hs:hs + E_LOC].reshape(NET, 128).T, dtype=f32)
    ones1 = np.ones((1, 128), dtype=bf16)
    ones8 = np.ones((128, H_LOC), dtype=f32)
    nmask = max(1, len(deltas))
    masks = np.zeros((128, nmask * 512), dtype=f32)
    jr = np.arange(128)[:, None]
    ir = np.arange(512)[None, :]
    for k, d in enumerate(deltas):
        masks[:, 512 * k:512 * (k + 1)] = (jr <= ir - d).astype(f32)
    return {"xqT": xqT, "xkvT": xkvT, "wqT": wqT, "wkT": wkT, "wvT": wvT,
            "woT": woT, "bq4": bq4, "bk4": bk4, "bv_row": bv_row,
            "ones1": ones1, "ones8": ones8, "masks": masks}


def kernel(query, key_value, Wq, bq, Wk, bk, Wv, bv, Wo, bo, lookahead,
           _trace=False):
    L = int(lookahead)
    if L not in _CACHE:
        _CACHE[L] = _build(L)
    nc, deltas = _CACHE[L]

    args = [np.asarray(a, dtype=np.float32) for a in
            (query, key_value, Wq, bq, Wk, bk, Wv, bv, Wo)]
    in_maps = [_prep_core(*args, c, deltas, L) for c in range(8)]
    res = run_bass_kernel_spmd(nc, in_maps, core_ids=list(range(8)),
                               trace=_trace)
    bo = np.asarray(bo, dtype=np.float32)
    out = np.empty((B, T, D), dtype=np.float32)
    for b in range(B):
        pT = res.results[2 * b]["outT"] + res.results[2 * b + 1]["outT"]
        out[b] = pT.T + bo[None, :]
    if _trace:
        kernel.last_exec_time_ns = res.exec_time_ns
    return out
